# revision 1
# baseline (speedup 1.0000x reference)
"""GCN encoder (2-layer GCNConv) as a Bass/Tile kernel on 8 Trainium2 NeuronCores.

Strategy (matches the sharding hint):
  - Nodes row-partitioned across 8 cores (6250 rows each); weights replicated.
  - Symmetric normalization factorized: z = D^-1/2 (A+I) D^-1/2 (x W) + b
    =>  u = dinv * (x W);  agg[d] = u[d] + sum_{e:dst=d} u[src_e];
        z = dinv * agg + b
    so no per-edge norm gather is needed.
  - Per layer: local matmul -> row scale -> AllGather(u) -> per-core gather of
    source rows (dma_gather) -> segment-sum via tensor-engine matmuls with
    compile-time-structured 0/1 selection matrices generated on DVE
    (is_equal against an iota) -> scale/bias/relu -> output rows.
  - Edges are bucketed host-side by (dst window of 128, src half) and padded to
    128-slot tiles; padded slots gather row 0 and have an all-zero selection
    column, so they contribute nothing.  int16 gather indices require the
    src-half split (indices < 32768).
"""

import math
import os
import sys

import numpy as np

sys.path.insert(0, "/opt/trn_rl_repo")

import ml_dtypes

BF16 = ml_dtypes.bfloat16


class Cfg:
    def __init__(self, N, E, IN=512, HID=256, OUT=128, P=8, half=None):
        self.N, self.E, self.IN, self.HID, self.OUT, self.P = N, E, IN, HID, OUT, P
        self.NC = N // P                      # nodes per core
        self.WS = 128                         # dst window size
        self.NW = math.ceil(self.NC / self.WS)  # windows per core
        # src-half split point (int16 gather indices must stay < 32768)
        if half is None:
            half = N if N <= 32767 else (N + 1) // 2
        self.HALF = half
        assert self.HALF <= 32767 and N - self.HALF <= 32767


FULL = Cfg(N=50000, E=800000)


def _prepare(cfg, x, edge_index, W1, b1, W2, b2):
    """Host-side graph preprocessing -> per-core input maps + program params."""
    N, P, NC, WS, NW, HALF = cfg.N, cfg.P, cfg.NC, cfg.WS, cfg.NW, cfg.HALF
    src = np.asarray(edge_index[0], dtype=np.int64)
    dst = np.asarray(edge_index[1], dtype=np.int64)

    deg = np.bincount(dst, minlength=N).astype(np.float64) + 1.0  # + self loop
    dinv = (1.0 / np.sqrt(deg)).astype(np.float32)

    # group id: ((core, window), src-half) ; groups contiguous after sort
    win_id = (dst // NC) * NW + (dst % NC) // WS
    half = (src >= HALF).astype(np.int64)
    comp = win_id * 2 + half
    order = np.argsort(comp, kind="stable")
    s_s, d_s, c_s = src[order], dst[order], comp[order]
    counts = np.bincount(c_s, minlength=P * NW * 2).reshape(P, NW, 2)

    # shared tile counts per (window, half): max over cores
    T = np.ceil(counts.max(axis=0) / 128).astype(np.int64)  # [NW, 2]
    tiles_total = int(T.sum())
    slots_total = tiles_total * 128

    starts = np.zeros(P * NW * 2 + 1, dtype=np.int64)
    np.cumsum(counts.reshape(-1), out=starts[1:])

    dinv_pad = np.concatenate(
        [dinv, np.ones(NW * WS * P - N, dtype=np.float32)])

    in_maps = []
    for c in range(P):
        idx_arr = np.zeros(slots_total, dtype=np.int16)
        aco_arr = np.full(slots_total, -1, dtype=np.float32)  # cast to bf16 below
        off = 0
        for w in range(NW):
            for h in range(2):
                g = (c * NW + w) * 2 + h
                n = counts[c, w, h]
                sl = slice(starts[g], starts[g] + n)
                idx_arr[off:off + n] = (s_s[sl] - h * HALF).astype(np.int16)
                aco_arr[off:off + n] = (d_s[sl] - c * NC - w * WS).astype(np.float32)
                off += 128 * int(T[w, h])
        assert off == slots_total

        dloc = np.concatenate(
            [dinv[c * NC:(c + 1) * NC],
             np.ones(NW * WS - NC, dtype=np.float32)])

        m = {
            "xT": np.ascontiguousarray(
                np.asarray(x[c * NC:(c + 1) * NC], np.float32).astype(BF16).T),
            "w1": np.ascontiguousarray(
                np.asarray(W1, np.float32).astype(BF16)
                .reshape(cfg.IN // 128, 128, cfg.HID).transpose(1, 0, 2)),
            "w2": np.ascontiguousarray(
                np.asarray(W2, np.float32).astype(BF16)
                .reshape(cfg.HID // 128, 128, cfg.OUT).transpose(1, 0, 2)),
            "dinvc": np.ascontiguousarray(dloc.reshape(NW, WS).T),
            "idx": np.ascontiguousarray(np.tile(idx_arr.reshape(-1, 16).T, (8, 1))),
            "acol": np.ascontiguousarray(aco_arr.reshape(-1, 128).T.astype(BF16)),
            "ident": np.eye(128, dtype=BF16),
        }
        b1nz = bool(np.any(np.asarray(b1)))
        b2nz = bool(np.any(np.asarray(b2)))
        if b1nz:
            m["b1bc"] = np.ascontiguousarray(
                np.broadcast_to(np.asarray(b1, np.float32), (128, cfg.HID)))
        if b2nz:
            m["b2bc"] = np.ascontiguousarray(
                np.broadcast_to(np.asarray(b2, np.float32), (128, cfg.OUT)))
        in_maps.append(m)

    return in_maps, T, b1nz, b2nz


def build_program(cfg, T, b1nz, b2nz):
    import concourse.bass as bass
    import concourse.bacc as bacc
    import concourse.mybir as mybir
    from concourse import tile

    N, P, NC, WS, NW = cfg.N, cfg.P, cfg.NC, cfg.WS, cfg.NW
    IN, HID, OUT = cfg.IN, cfg.HID, cfg.OUT
    NCI, NCH = IN // 128, HID // 128
    tiles_total = int(T.sum())
    slots_total = tiles_total * 128
    f32, bf16, i16 = mybir.dt.float32, mybir.dt.bfloat16, mybir.dt.int16
    AF = mybir.ActivationFunctionType

    nc = bacc.Bacc("TRN2", target_bir_lowering=False, debug=False,
                   num_devices=cfg.P)
    xT_p = nc.dram_tensor("xT", [IN, NC], bf16, kind="ExternalInput")
    w1_p = nc.dram_tensor("w1", [128, NCI, HID], bf16, kind="ExternalInput")
    w2_p = nc.dram_tensor("w2", [128, NCH, OUT], bf16, kind="ExternalInput")
    dinv_p = nc.dram_tensor("dinvc", [WS, NW], f32, kind="ExternalInput")
    idx_p = nc.dram_tensor("idx", [128, slots_total // 16], i16, kind="ExternalInput")
    acol_p = nc.dram_tensor("acol", [128, tiles_total], bf16, kind="ExternalInput")
    id_p = nc.dram_tensor("ident", [128, 128], bf16, kind="ExternalInput")
    b1_p = (nc.dram_tensor("b1bc", [128, HID], f32, kind="ExternalInput")
            if b1nz else None)
    b2_p = (nc.dram_tensor("b2bc", [128, OUT], f32, kind="ExternalInput")
            if b2nz else None)
    out_p = nc.dram_tensor("out", [NC, OUT], f32, kind="ExternalOutput")

    u1d = nc.dram_tensor("u1d", [NC, HID], bf16)
    u2d = nc.dram_tensor("u2d", [NC, OUT], bf16)
    U1 = nc.dram_tensor("U1", [N, HID], bf16)
    U2 = nc.dram_tensor("U2", [N, OUT], bf16)
    rg = [list(range(P))]

    with tile.TileContext(nc) as tc:
        with (
            tc.tile_pool(name="res", bufs=1) as res,
            tc.tile_pool(name="work", bufs=4) as work,
            tc.tile_pool(name="gath", bufs=4) as gath,
            tc.tile_pool(name="psum", bufs=2, space="PSUM") as psum,
        ):
            # ---- resident loads ----
            xTs = res.tile([128, NCI, NC], bf16)
            for ci in range(NCI):
                nc.sync.dma_start(xTs[:, ci, :], xT_p[ci * 128:(ci + 1) * 128, :])
            w1s = res.tile([128, NCI, HID], bf16)
            nc.sync.dma_start(w1s[:], w1_p[:])
            w2s = res.tile([128, NCH, OUT], bf16)
            nc.sync.dma_start(w2s[:], w2_p[:])
            dinvs = res.tile([WS, NW], f32)
            nc.sync.dma_start(dinvs[:], dinv_p[:])
            idxs = res.tile([128, slots_total // 16], i16)
            nc.sync.dma_start(idxs[:], idx_p[:])
            acols = res.tile([128, tiles_total], bf16)
            nc.sync.dma_start(acols[:], acol_p[:])
            ident = res.tile([128, 128], bf16)
            nc.sync.dma_start(ident[:], id_p[:])
            iot = res.tile([128, 128], bf16)
            nc.gpsimd.iota(iot[:], pattern=[[1, 128]], base=0,
                           channel_multiplier=0,
                           allow_small_or_imprecise_dtypes=True)
            b1bc = None
            if b1nz:
                b1bc = res.tile([128, HID], f32)
                nc.sync.dma_start(b1bc[:], b1_p[:])
            b2bc = None
            if b2nz:
                b2bc = res.tile([128, OUT], f32)
                nc.sync.dma_start(b2bc[:], b2_p[:])

            u1res = res.tile([128, NW, HID], bf16)
            u2res = res.tile([128, NW, OUT], bf16)
            h1T = res.tile([128, NCH, NC], bf16)
            if NC % WS:
                # tail rows of the last window feed the self-loop matmul as
                # rhs; zero them so uninitialized SBUF can't inject NaNs
                nc.gpsimd.memset(u1res[:, NW - 1, :], 0.0)
                nc.gpsimd.memset(u2res[:, NW - 1, :], 0.0)

            def nsz(j):
                return min(128, NC - j * WS)

            MAXP = int(os.environ.get("GCN_MAX_PHASE", "9"))

            def emit_debug_out(src_bf16_ap, w, n):
                # convert [n, OUT] bf16 -> f32, dump into out rows of window w
                dt = work.tile([128, OUT], f32, tag="dbg")
                nc.scalar.activation(dt[:n, :], src_bf16_ap, AF.Copy)
                nc.sync.dma_start(out_p[w * WS:w * WS + n, :], dt[:n, :])

            # ---- phase A: t1 = x @ W1 ; u1 = dinv * t1 ----
            for j in range(NW):
                n = nsz(j)
                jsl = slice(j * WS, j * WS + n)
                pt = psum.tile([128, HID], f32, tag="mm")
                for ci in range(NCI):
                    nc.tensor.matmul(pt[:n, :], xTs[:, ci, jsl],
                                     w1s[:, ci, :], start=(ci == 0),
                                     stop=(ci == NCI - 1))
                nc.scalar.activation(u1res[:n, j, :], pt[:n, :], AF.Copy,
                                     scale=dinvs[:n, j:j + 1])
                nc.sync.dma_start(u1d[jsl, :], u1res[:n, j, :])
                if MAXP == 1:
                    emit_debug_out(u1res[:n, j, :OUT], j, n)
            if MAXP <= 1:
                return nc

            # ---- AllGather u1 ----
            nc.gpsimd.collective_compute(
                "AllGather", mybir.AluOpType.bypass, replica_groups=rg,
                ins=[u1d[:]], outs=[U1[:]])
            if MAXP == 2:
                for j in range(NW):
                    n = nsz(j)
                    gt = work.tile([128, OUT], bf16, tag="dbg_g")
                    nc.sync.dma_start(gt[:n, :], U1[j * WS:j * WS + n, :OUT])
                    emit_debug_out(gt[:n, :], j, n)
                return nc

            # ---- generic aggregation layer ----
            def agg_layer(U, F, ures, bbc, relu, emit_out):
                tile_idx = 0
                slot_off = 0
                for w in range(NW):
                    n = nsz(w)
                    pa = psum.tile([128, F], f32, tag="agg")
                    # self-loop term: ident.T @ u[w]
                    nc.tensor.matmul(pa[:n, :], ident[:, :n], ures[:, w, :],
                                     start=True, stop=False)
                    nmm = int(T[w, 0] + T[w, 1])
                    done = 0
                    for h in range(2):
                        t_wh = int(T[w, h])
                        if t_wh == 0:
                            continue
                        g = gath.tile([128, t_wh, F], bf16, tag="g")
                        base = 0 if h == 0 else cfg.HALF
                        nc.gpsimd.dma_gather(
                            g[:], U[base:base + min(cfg.HALF, N - base), :],
                            idxs[:, slot_off // 16:
                                 (slot_off + 128 * t_wh) // 16],
                            num_idxs=128 * t_wh, num_idxs_reg=128 * t_wh,
                            elem_size=F, single_packet=False)
                        slot_off += 128 * t_wh
                        for t in range(t_wh):
                            S = work.tile([128, 128], bf16, tag="S")
                            nc.vector.tensor_tensor(
                                S[:], iot[:],
                                acols[:, tile_idx:tile_idx + 1]
                                .broadcast_to((128, 128)),
                                op=mybir.AluOpType.is_equal)
                            tile_idx += 1
                            done += 1
                            nc.tensor.matmul(pa[:n, :], S[:, :n], g[:, t, :],
                                             start=False, stop=(done == nmm))
                    # z = dinv * agg (+ b) ; relu
                    if bbc is None:
                        zf = AF.Relu if relu else AF.Copy
                        zt = work.tile([128, F], f32 if emit_out else bf16,
                                       tag="zt%d" % F)
                        nc.scalar.activation(zt[:n, :], pa[:n, :], zf,
                                             scale=dinvs[:n, w:w + 1])
                    else:
                        v = work.tile([128, F], f32, tag="v%d" % F)
                        nc.scalar.activation(v[:n, :], pa[:n, :], AF.Copy,
                                             scale=dinvs[:n, w:w + 1])
                        zt = work.tile([128, F], f32 if emit_out else bf16,
                                       tag="zt%d" % F)
                        if relu:
                            vb = work.tile([128, F], f32, tag="vb%d" % F)
                            nc.vector.tensor_tensor(
                                vb[:n, :], v[:n, :], bbc[:n, :],
                                op=mybir.AluOpType.add)
                            nc.scalar.activation(zt[:n, :], vb[:n, :], AF.Relu)
                        else:
                            nc.vector.tensor_tensor(
                                zt[:n, :], v[:n, :], bbc[:n, :],
                                op=mybir.AluOpType.add)
                    yield w, n, zt

            # ---- phase C: layer-1 aggregation -> h1 -> h1T ----
            for w, n, zt in agg_layer(U1, HID, u1res, b1bc, True, False):
                wsl = slice(w * WS, w * WS + n)
                for ch in range(NCH):
                    ptr = psum.tile([128, 128], bf16, tag="tr")
                    nc.tensor.transpose(ptr[:, :n],
                                        zt[:n, ch * 128:(ch + 1) * 128],
                                        ident[:n, :n])
                    nc.scalar.activation(h1T[:, ch, wsl], ptr[:, :n], AF.Copy)
                if MAXP == 3:
                    emit_debug_out(zt[:n, :OUT], w, n)
            if MAXP <= 3:
                return nc

            # ---- phase D: t2 = h1 @ W2 ; u2 ----
            for j in range(NW):
                n = nsz(j)
                jsl = slice(j * WS, j * WS + n)
                pt = psum.tile([128, OUT], f32, tag="mm")
                for ch in range(NCH):
                    nc.tensor.matmul(pt[:n, :], h1T[:, ch, jsl],
                                     w2s[:, ch, :], start=(ch == 0),
                                     stop=(ch == NCH - 1))
                nc.scalar.activation(u2res[:n, j, :], pt[:n, :], AF.Copy,
                                     scale=dinvs[:n, j:j + 1])
                nc.sync.dma_start(u2d[jsl, :], u2res[:n, j, :])
                if MAXP == 4:
                    emit_debug_out(u2res[:n, j, :], j, n)
            if MAXP <= 4:
                return nc

            # ---- AllGather u2 ----
            nc.gpsimd.collective_compute(
                "AllGather", mybir.AluOpType.bypass, replica_groups=rg,
                ins=[u2d[:]], outs=[U2[:]])

            # ---- phase F: layer-2 aggregation -> out ----
            for w, n, zt in agg_layer(U2, OUT, u2res, b2bc, False, True):
                wsl = slice(w * WS, w * WS + n)
                nc.sync.dma_start(out_p[wsl, :], zt[:n, :])

    return nc


def run(cfg, inputs, sim=False, trace=False):
    from concourse.bass_utils import run_bass_kernel_spmd

    in_maps, T, b1nz, b2nz = _prepare(
        cfg, inputs["x"], inputs["edge_index"], inputs["W1"], inputs["b1"],
        inputs["W2"], inputs["b2"])
    nc = build_program(cfg, T, b1nz, b2nz)
    nc.finalize()
    core_ids = list(range(cfg.P))
    if sim:
        from concourse import bass_interp
        ms = bass_interp.MultiCoreSim(nc, cfg.P)
        for c in core_ids:
            for k, v in in_maps[c].items():
                ms.cores[c].tensor(k)[:] = v
        ms.simulate()
        outs = [np.array(ms.cores[c].tensor("out")) for c in core_ids]
        return np.concatenate(outs, axis=0), None
    res = run_bass_kernel_spmd(nc, in_maps, core_ids, trace=trace)
    outs = [np.asarray(res.results[c]["out"]) for c in core_ids]
    return np.concatenate(outs, axis=0), res


def kernel(x, edge_index, W1, b1, W2, b2):
    out, _ = run(FULL, dict(x=x, edge_index=edge_index, W1=W1, b1=b1,
                            W2=W2, b2=b2))
    return out



# revision 15
# speedup vs baseline: 2.2186x; 2.2186x over previous
"""GCN encoder (2-layer GCNConv) as a Bass/Tile kernel on 8 Trainium2 NeuronCores.

Strategy (matches the sharding hint):
  - Nodes row-partitioned across 8 cores (6250 rows each); weights replicated.
  - Symmetric normalization factorized: z = D^-1/2 (A+I) D^-1/2 (x W) + b
    =>  u = dinv * (x W);  agg[d] = u[d] + sum_{e:dst=d} u[src_e];
        z = dinv * agg + b
  - Per layer: local matmul -> row scale -> AllGather(u) -> per-core gather of
    source rows (dma_gather) -> segment-sum via tensor-engine matmuls with
    0/1 selection matrices generated on DVE -> scale/bias/relu.
  - Edges bucketed host-side by (dst window of 128, source class) and padded
    to 128-slot tiles; padded slots have an all-zero selection column.

Perf notes:
  - dma_gather descriptor throughput is the bottleneck (~1.5ns/row fixed +
    ~4.3ps/B aggregate over the 4 SWDGE queues; a single queue only sustains
    ~8-9ns/row).  Gathers use a greedy least-loaded queue assignment.
  - AllGather outputs are addr_space="Shared" (the fast collective path).
    Collectives pay ~50-80us re-arm between closely-spaced ops, and the
    AllGather itself is HBM-bound (overlapping local gather traffic with it
    just steals its bandwidth - measured, not guessed).  So: layer 1 uses one
    AllGather (global-half gather tables); layer 2's is split in two
    row-chunks, chunk A's AllGather triggered mid-layer-1 (hidden), chunk B's
    at its end behind chunk-A lead gathers.  The class-0 ("own core source")
    machinery below is kept but disabled for that reason.
  - x^T is streamed per window; loads alternate between two engine DMA
    queues so phase A is not issue-limited.
  - Selection matrices: one batched DVE is_equal per bucket, emitted two
    windows ahead of consumption.
  - Per-window u1res/u2res tiles keep dependency tracking fine-grained.
  - Layer-2 transform (h1 @ W2) is pipelined into the layer-1 aggregation
    loop one window behind.
"""

import math
import sys

import numpy as np

sys.path.insert(0, "/opt/trn_rl_repo")

import ml_dtypes

BF16 = ml_dtypes.bfloat16

NQ = 4     # SWDGE queues (ucode MAX_SWDGE_QUEUES)
LEAD1 = 7  # class-1 gather lead (windows)
LEAD2 = 3  # class-2 gather lead (windows)
SLEAD = 2  # selection-matrix generation lead (windows)


class Cfg:
    def __init__(self, N, E, IN=512, HID=256, OUT=128, P=8):
        self.N, self.E, self.IN, self.HID, self.OUT, self.P = N, E, IN, HID, OUT, P
        self.NC = N // P                      # nodes per core
        self.WS = 128                         # dst window size
        self.NW = math.ceil(self.NC / self.WS)  # windows per core
        # layer-1 table split: global halves (int16 indices)
        self.HALF = N if N <= 32767 else (N + 1) // 2
        assert self.HALF <= 32767 and N - self.HALF <= 32767
        # layer-2 table split: row-chunks per core (A = first AW windows)
        self.AW = max(1, min(self.NW - 1, math.ceil(self.NW / 3)))
        self.RH = self.AW * self.WS           # chunk-A rows per core
        self.NCB = self.NC - self.RH          # chunk-B rows per core
        assert self.P * self.RH <= 32768 and self.P * self.NCB <= 32768
        assert self.NC <= 32767


FULL = Cfg(N=50000, E=800000)


def _bucketize(cfg, src, dst, cls, tidx):
    """Sort edges into (dst-window, source-class) buckets; shared tile counts."""
    P, NC, WS, NW = cfg.P, cfg.NC, cfg.WS, cfg.NW
    win_id = (dst // NC) * NW + (dst % NC) // WS
    comp = win_id * 3 + cls
    order = np.argsort(comp, kind="stable")
    t_s, d_s, c_s = tidx[order], dst[order], comp[order]
    counts = np.bincount(c_s, minlength=P * NW * 3).reshape(P, NW, 3)
    T = np.ceil(counts.max(axis=0) / 128).astype(np.int64)  # [NW, 3]
    starts = np.zeros(P * NW * 3 + 1, dtype=np.int64)
    np.cumsum(counts.reshape(-1), out=starts[1:])
    return t_s, d_s, counts, T, starts


def _fill_slots(cfg, c, counts, T, starts, t_s, d_s, idx_arr, aco_arr, off0):
    NW = cfg.NW
    off = off0
    for k in range(3):
        for w in range(NW):
            g = (c * NW + w) * 3 + k
            n = counts[c, w, k]
            sl = slice(starts[g], starts[g] + n)
            idx_arr[off:off + n] = t_s[sl].astype(np.int16)
            aco_arr[off:off + n] = (d_s[sl] - c * cfg.NC - w * cfg.WS).astype(
                np.float32)
            off += 128 * int(T[w, k])
    return off


def _prepare(cfg, x, edge_index, W1, b1, W2, b2):
    """Host-side graph preprocessing -> per-core input maps + program params."""
    N, P, NC, WS, NW = cfg.N, cfg.P, cfg.NC, cfg.WS, cfg.NW
    RH, NCB, HALF = cfg.RH, cfg.NCB, cfg.HALF
    src = np.asarray(edge_index[0], dtype=np.int64)
    dst = np.asarray(edge_index[1], dtype=np.int64)

    deg = np.bincount(dst, minlength=N).astype(np.float64) + 1.0  # + self loop
    dinv = (1.0 / np.sqrt(deg)).astype(np.float32)

    s_core, s_row = src // NC, src % NC
    own = (s_core == dst // NC)
    # layer 1: class 0 = own core (local u1d), 1/2 = global halves of U1
    h1 = (src >= HALF).astype(np.int64)
    cls1 = np.where(own, 0, 1 + h1)
    ti1 = np.where(own, s_row, src - h1 * HALF)
    t_s1, d_s1, counts1, T1, starts1 = _bucketize(cfg, src, dst, cls1, ti1)
    # layer 2: class 0 = own core (local u2dL), 1/2 = row-chunk tables
    h2 = (s_row >= RH).astype(np.int64)
    cls2 = np.where(own, 0, 1 + h2)
    ti2 = np.where(own, s_row,
                   np.where(h2 == 0, s_core * RH + s_row,
                            s_core * NCB + (s_row - RH)))
    t_s2, d_s2, counts2, T2, starts2 = _bucketize(cfg, src, dst, cls2, ti2)

    slots_total = int(T1.sum() + T2.sum()) * 128

    in_maps = []
    for c in range(P):
        idx_arr = np.zeros(slots_total, dtype=np.int16)
        aco_arr = np.full(slots_total, -1, dtype=np.float32)
        off = _fill_slots(cfg, c, counts1, T1, starts1, t_s1, d_s1,
                          idx_arr, aco_arr, 0)
        off = _fill_slots(cfg, c, counts2, T2, starts2, t_s2, d_s2,
                          idx_arr, aco_arr, off)
        assert off == slots_total

        dloc = np.concatenate(
            [dinv[c * NC:(c + 1) * NC],
             np.ones(NW * WS - NC, dtype=np.float32)])

        # x^T in per-window layout [128, NW, NCI, 128]:
        # xTw[p, j, ci, col] = x[row j*128+col, ci*128+p]
        NCI = cfg.IN // 128
        xp = np.zeros((NW * WS, cfg.IN), dtype=np.float32)
        xp[:NC] = np.asarray(x[c * NC:(c + 1) * NC], np.float32)
        xTw = np.ascontiguousarray(
            xp.astype(BF16).reshape(NW, WS, NCI, 128).transpose(3, 0, 2, 1))

        m = {
            "xTw": xTw,
            "w1": np.ascontiguousarray(
                np.asarray(W1, np.float32).astype(BF16)
                .reshape(cfg.IN // 128, 128, cfg.HID).transpose(1, 0, 2)),
            "w2": np.ascontiguousarray(
                np.asarray(W2, np.float32).astype(BF16)
                .reshape(cfg.HID // 128, 128, cfg.OUT).transpose(1, 0, 2)),
            "dinvc": np.ascontiguousarray(dloc.reshape(NW, WS).T),
            "idx": np.ascontiguousarray(np.tile(idx_arr.reshape(-1, 16).T, (8, 1))),
            "acol": np.ascontiguousarray(aco_arr.reshape(-1, 128).T.astype(BF16)),
            "ident": np.eye(128, dtype=BF16),
        }
        b1nz = bool(np.any(np.asarray(b1)))
        b2nz = bool(np.any(np.asarray(b2)))
        if b1nz:
            m["b1bc"] = np.ascontiguousarray(
                np.broadcast_to(np.asarray(b1, np.float32), (128, cfg.HID)))
        if b2nz:
            m["b2bc"] = np.ascontiguousarray(
                np.broadcast_to(np.asarray(b2, np.float32), (128, cfg.OUT)))
        in_maps.append(m)

    return in_maps, T1, T2, b1nz, b2nz


def build_program(cfg, T1, T2, b1nz, b2nz):
    import concourse.bacc as bacc
    import concourse.mybir as mybir
    from concourse import tile

    N, P, NC, WS, NW = cfg.N, cfg.P, cfg.NC, cfg.WS, cfg.NW
    AW, RH, NCB, HALF = cfg.AW, cfg.RH, cfg.NCB, cfg.HALF
    IN, HID, OUT = cfg.IN, cfg.HID, cfg.OUT
    NCI, NCH = IN // 128, HID // 128
    tiles_total = int(T1.sum() + T2.sum())
    slots_total = tiles_total * 128
    # gather/selection ring slot sizes: max over layers per class
    GB = [max(int(T1[:, k].max()) * HID, int(T2[:, k].max()) * OUT)
          for k in range(3)]  # bf16 elements per partition
    # own-class gathers are merged over 4-window groups
    GB[0] = max(max(int(T1[w:w + 4, 0].sum()) for w in range(0, NW, 4)) * HID,
                max(int(T2[w:w + 4, 0].sum()) for w in range(0, NW, 4)) * OUT)
    SM = [max(int(T1[:, k].max()), int(T2[:, k].max())) for k in range(3)]

    def offsets(T, base_tiles):
        so = np.zeros((NW, 3), dtype=np.int64)
        to = np.zeros((NW, 3), dtype=np.int64)
        acc = base_tiles
        for k in range(3):
            for w in range(NW):
                to[w, k] = acc
                so[w, k] = acc * 128
                acc += int(T[w, k])
        return so, to, acc

    so1, to1, acc1 = offsets(T1, 0)
    so2, to2, _ = offsets(T2, acc1)

    f32, bf16, i16 = mybir.dt.float32, mybir.dt.bfloat16, mybir.dt.int16
    AF = mybir.ActivationFunctionType

    nc = bacc.Bacc("TRN2", target_bir_lowering=False, debug=False,
                   num_devices=cfg.P, num_swdge_queues=NQ)
    xTw_p = nc.dram_tensor("xTw", [128, NW, NCI, 128], bf16, kind="ExternalInput")
    w1_p = nc.dram_tensor("w1", [128, NCI, HID], bf16, kind="ExternalInput")
    w2_p = nc.dram_tensor("w2", [128, NCH, OUT], bf16, kind="ExternalInput")
    dinv_p = nc.dram_tensor("dinvc", [WS, NW], f32, kind="ExternalInput")
    idx_p = nc.dram_tensor("idx", [128, slots_total // 16], i16, kind="ExternalInput")
    acol_p = nc.dram_tensor("acol", [128, tiles_total], bf16, kind="ExternalInput")
    id_p = nc.dram_tensor("ident", [128, 128], bf16, kind="ExternalInput")
    b1_p = (nc.dram_tensor("b1bc", [128, HID], f32, kind="ExternalInput")
            if b1nz else None)
    b2_p = (nc.dram_tensor("b2bc", [128, OUT], f32, kind="ExternalInput")
            if b2nz else None)
    out_p = nc.dram_tensor("out", [NC, OUT], f32, kind="ExternalOutput")

    u1d = nc.dram_tensor("u1d", [NC, HID], bf16)
    u2dL = nc.dram_tensor("u2dL", [NC, OUT], bf16)
    u2dA = nc.dram_tensor("u2dA", [RH, OUT], bf16)
    u2dB = nc.dram_tensor("u2dB", [NCB, OUT], bf16)
    U1 = nc.dram_tensor("U1", [N, HID], bf16, addr_space="Shared")
    U2a = nc.dram_tensor("U2a", [P * RH, OUT], bf16, addr_space="Shared")
    U2b = nc.dram_tensor("U2b", [P * NCB, OUT], bf16, addr_space="Shared")
    rg = [list(range(P))]

    with tile.TileContext(nc) as tc:
        with (
            tc.tile_pool(name="res", bufs=1) as res,
            tc.tile_pool(name="xa", bufs=6) as xapool,
            tc.tile_pool(name="work", bufs=4) as work,
            tc.tile_pool(name="g0", bufs=3) as g0pool,
            tc.tile_pool(name="g1", bufs=LEAD1 + 2) as g1pool,
            tc.tile_pool(name="g2", bufs=LEAD2 + 2) as g2pool,
            tc.tile_pool(name="s0", bufs=4) as s0pool,
            tc.tile_pool(name="s1", bufs=SLEAD + 2) as s1pool,
            tc.tile_pool(name="s2", bufs=SLEAD + 2) as s2pool,
            tc.tile_pool(name="psum", bufs=4, space="PSUM") as psum,
            tc.tile_pool(name="psum2", bufs=2, space="PSUM") as psum2,
        ):
            # ---- resident loads ----
            w1s = res.tile([128, NCI, HID], bf16)
            nc.sync.dma_start(w1s[:], w1_p[:])
            w2s = res.tile([128, NCH, OUT], bf16)
            nc.sync.dma_start(w2s[:], w2_p[:])
            dinvs = res.tile([WS, NW], f32)
            nc.sync.dma_start(dinvs[:], dinv_p[:])
            idxs = res.tile([128, slots_total // 16], i16)
            nc.sync.dma_start(idxs[:], idx_p[:])
            acols = res.tile([128, tiles_total], bf16)
            nc.sync.dma_start(acols[:], acol_p[:])
            ident = res.tile([128, 128], bf16)
            nc.sync.dma_start(ident[:], id_p[:])
            iot = res.tile([128, 128], bf16)
            nc.gpsimd.iota(iot[:], pattern=[[1, 128]], base=0,
                           channel_multiplier=0,
                           allow_small_or_imprecise_dtypes=True)
            b1bc = None
            if b1nz:
                b1bc = res.tile([128, HID], f32)
                nc.sync.dma_start(b1bc[:], b1_p[:])
            b2bc = None
            if b2nz:
                b2bc = res.tile([128, OUT], f32)
                nc.sync.dma_start(b2bc[:], b2_p[:])

            u1res = [res.tile([128, HID], bf16) for _ in range(NW)]
            u2res = [res.tile([128, OUT], bf16) for _ in range(NW)]
            if NC % WS:
                nc.gpsimd.memset(u1res[NW - 1][:], 0.0)
                nc.gpsimd.memset(u2res[NW - 1][:], 0.0)

            def nsz(j):
                return min(128, NC - j * WS)

            # ---- phase A: t1 = x @ W1 ; u1 = dinv * t1 ----
            load_engs = [nc.sync, nc.scalar]
            for j in range(NW):
                n = nsz(j)
                xa = xapool.tile([128, NCI, 128], bf16, tag="xa")
                load_engs[j % 2].dma_start(xa[:], xTw_p[:, j, :, :])
                pt = psum2.tile([128, HID], f32, tag="mm")
                for ci in range(NCI):
                    nc.tensor.matmul(pt[:n, :], xa[:, ci, :n],
                                     w1s[:, ci, :], start=(ci == 0),
                                     stop=(ci == NCI - 1))
                nc.scalar.activation(u1res[j][:n, :], pt[:n, :], AF.Copy,
                                     scale=dinvs[:n, j:j + 1])
                nc.sync.dma_start(u1d[j * WS:j * WS + n, :], u1res[j][:n, :])
            nc.gpsimd.collective_compute(
                "AllGather", mybir.AluOpType.bypass, replica_groups=rg,
                ins=[u1d[:]], outs=[U1[:]])

            qload = [0, 0, 0, 0]
            pools = [g0pool, g1pool, g2pool]
            spools = [s0pool, s1pool, s2pool]

            def emit_gather(tables, T, so, F, w, k):
                """Issue the (w, k) bucket's gather."""
                t_wk = int(T[w, k])
                if t_wk == 0:
                    return None
                g = pools[k].tile([128, GB[k]], bf16, tag="g%d" % k)
                gv = g[:, :t_wk * F].rearrange("p (t f) -> p t f", f=F)
                so_wk = int(so[w, k])
                q = min(range(NQ), key=lambda i: qload[i])
                qload[q] += t_wk
                nc.gpsimd.dma_gather(
                    gv, tables[k],
                    idxs[:, so_wk // 16:(so_wk + 128 * t_wk) // 16],
                    num_idxs=128 * t_wk, num_idxs_reg=128 * t_wk,
                    elem_size=F, single_packet=False, queue_num=q)
                return gv

            def emit_sgen(T, to, w, k):
                t_wk = int(T[w, k])
                if t_wk == 0:
                    return None
                ti = int(to[w, k])
                S = spools[k].tile([128, SM[k], 128], bf16, tag="S%d" % k)
                nc.vector.tensor_tensor(
                    S[:, :t_wk, :],
                    iot[:, None, :].broadcast_to((128, t_wk, 128)),
                    acols[:, ti:ti + t_wk, None]
                    .broadcast_to((128, t_wk, 128)),
                    op=mybir.AluOpType.is_equal)
                return S

            def own_phase(table, T, so, to, F, ures):
                """Aggregate own-core edges into ures during the AllGather.

                Gathers are merged over OG-window groups (class-major slot
                layout makes their slots contiguous) to amortize per-call
                overhead; each window keeps its own ures tile so the
                read-modify-write pipelines window to window."""
                OG = 4
                for w0 in range(0, NW, OG):
                    ws = range(w0, min(w0 + OG, NW))
                    tg = sum(int(T[w, 0]) for w in ws)
                    if tg == 0:
                        continue
                    g = g0pool.tile([128, GB[0]], bf16, tag="g0")
                    gv = g[:, :tg * F].rearrange("p (t f) -> p t f", f=F)
                    so_g = int(so[w0, 0])
                    q = min(range(NQ), key=lambda i: qload[i])
                    qload[q] += tg
                    nc.gpsimd.dma_gather(
                        gv, table,
                        idxs[:, so_g // 16:(so_g + 128 * tg) // 16],
                        num_idxs=128 * tg, num_idxs_reg=128 * tg,
                        elem_size=F, single_packet=False, queue_num=q)
                    for w in ws:
                        n = nsz(w)
                        t_w = int(T[w, 0])
                        if t_w == 0:
                            continue
                        toff = int(to[w, 0]) - int(to[w0, 0])
                        S = emit_sgen(T, to, w, 0)
                        pa = psum.tile([128, F], f32, tag="agg")
                        nc.tensor.matmul(pa[:n, :], ident[:, :n], ures[w][:],
                                         start=True, stop=False)
                        for t in range(t_w):
                            nc.tensor.matmul(
                                pa[:n, :], S[:, t, :n], gv[:, toff + t, :],
                                start=False, stop=(t == t_w - 1))
                        nc.scalar.activation(ures[w][:n, :], pa[:n, :],
                                             AF.Copy)

            def agg_matmuls(w, T, F, ures, gbufs, sbufs):
                n = nsz(w)
                pa = psum.tile([128, F], f32, tag="agg")
                nc.tensor.matmul(pa[:n, :], ident[:, :n], ures[w][:],
                                 start=True, stop=False)
                nmm = int(T[w, 1] + T[w, 2])
                done = 0
                for k in (1, 2):
                    if gbufs.get(k) is None:
                        continue
                    g, S = gbufs[k], sbufs[k]
                    for t in range(int(T[w, k])):
                        done += 1
                        nc.tensor.matmul(pa[:n, :], S[:, t, :n], g[:, t, :],
                                         start=False, stop=(done == nmm))
                return pa, n

            def agg_layer(tables, T, so, to, F, ures, window_cb,
                          post_win_cb=None, mid_hook=None):
                gb = {}
                sb = {}
                for w in range(min(LEAD1, NW)):
                    gb.setdefault(w, {})[1] = emit_gather(tables, T, so, F, w, 1)
                if mid_hook is not None:
                    mid_hook()
                for w in range(min(LEAD2, NW)):
                    gb.setdefault(w, {})[2] = emit_gather(tables, T, so, F, w, 2)
                for w in range(min(SLEAD, NW)):
                    sb[w] = {1: emit_sgen(T, to, w, 1), 2: emit_sgen(T, to, w, 2)}
                for w in range(NW):
                    if w + LEAD1 < NW:
                        gb.setdefault(w + LEAD1, {})[1] = emit_gather(
                            tables, T, so, F, w + LEAD1, 1)
                    if w + LEAD2 < NW:
                        gb.setdefault(w + LEAD2, {})[2] = emit_gather(
                            tables, T, so, F, w + LEAD2, 2)
                    if w + SLEAD < NW:
                        sb[w + SLEAD] = {1: emit_sgen(T, to, w + SLEAD, 1),
                                         2: emit_sgen(T, to, w + SLEAD, 2)}
                    pa, n = agg_matmuls(w, T, F, ures, gb.pop(w), sb.pop(w))
                    window_cb(pa, n, w)
                    if post_win_cb is not None:
                        post_win_cb(w)

            def finish1(pa, n, w):
                zt = work.tile([128, HID], bf16, tag="zt1")
                if b1bc is None:
                    nc.scalar.activation(zt[:n, :], pa[:n, :], AF.Relu,
                                         scale=dinvs[:n, w:w + 1])
                else:
                    v = work.tile([128, HID], f32, tag="v1")
                    nc.scalar.activation(v[:n, :], pa[:n, :], AF.Copy,
                                         scale=dinvs[:n, w:w + 1])
                    vb = work.tile([128, HID], f32, tag="vb1")
                    nc.vector.tensor_tensor(vb[:n, :], v[:n, :], b1bc[:n, :],
                                            op=mybir.AluOpType.add)
                    nc.scalar.activation(zt[:n, :], vb[:n, :], AF.Relu)
                return zt

            def phaseD(zt, w):
                n = nsz(w)
                h1tw = work.tile([128, NCH, 128], bf16, tag="h1t")
                for ch in range(NCH):
                    ptr = psum2.tile([128, 128], bf16, tag="tr")
                    nc.tensor.transpose(ptr[:, :n],
                                        zt[:n, ch * 128:(ch + 1) * 128],
                                        ident[:n, :n])
                    nc.scalar.activation(h1tw[:, ch, :n], ptr[:, :n], AF.Copy)
                pt = psum2.tile([128, OUT], f32, tag="mm")
                for ch in range(NCH):
                    nc.tensor.matmul(pt[:n, :], h1tw[:, ch, :n],
                                     w2s[:, ch, :], start=(ch == 0),
                                     stop=(ch == NCH - 1))
                nc.scalar.activation(u2res[w][:n, :], pt[:n, :], AF.Copy,
                                     scale=dinvs[:n, w:w + 1])
                j0 = w * WS
                nc.sync.dma_start(u2dL[j0:j0 + n, :], u2res[w][:n, :])
                if w < AW:
                    nc.scalar.dma_start(u2dA[j0:j0 + n, :], u2res[w][:n, :])
                else:
                    nc.scalar.dma_start(u2dB[j0 - RH:j0 - RH + n, :],
                                        u2res[w][:n, :])

            # ---- layer 1: own-class aggregation during the AllGather ----
            own_phase(u1d[:], T1, so1, to1, HID, u1res)

            # ---- layer-1 aggregation, phase D pipelined one window back ----
            pending = [None]

            def l1_cb(pa, n, w):
                if pending[0] is not None:
                    phaseD(*pending[0])
                pending[0] = (finish1(pa, n, w), w)

            def l1_post(w):
                # phaseD(AW-1) ran during window AW -> u2dA complete; chunk-A
                # AllGather runs hidden inside layer-1 aggregation.
                if w == AW:
                    nc.gpsimd.collective_compute(
                        "AllGather", mybir.AluOpType.bypass, replica_groups=rg,
                        ins=[u2dA[:]], outs=[U2a[:]])

            agg_layer((None, U1[0:HALF, :], U1[HALF:N, :]), T1, so1, to1,
                      HID, u1res, l1_cb, l1_post if NW > AW else None)
            phaseD(*pending[0])
            if NW <= AW:
                nc.gpsimd.collective_compute(
                    "AllGather", mybir.AluOpType.bypass, replica_groups=rg,
                    ins=[u2dA[:]], outs=[U2a[:]])

            def l2_mid():
                # chunk-B AllGather + own-class work run while chunk-A lead
                # gathers drain; emitted after those leads so they are not
                # stuck behind this trigger's wait on the u2dB stores.
                nc.gpsimd.collective_compute(
                    "AllGather", mybir.AluOpType.bypass, replica_groups=rg,
                    ins=[u2dB[:]], outs=[U2b[:]])
                own_phase(u2dL[:], T2, so2, to2, OUT, u2res)

            # ---- layer-2 aggregation -> out ----
            def l2_cb(pa, n, w):
                zt = work.tile([128, OUT], f32, tag="zt2")
                if b2bc is None:
                    nc.scalar.activation(zt[:n, :], pa[:n, :], AF.Copy,
                                         scale=dinvs[:n, w:w + 1])
                else:
                    v = work.tile([128, OUT], f32, tag="v2")
                    nc.scalar.activation(v[:n, :], pa[:n, :], AF.Copy,
                                         scale=dinvs[:n, w:w + 1])
                    nc.vector.tensor_tensor(zt[:n, :], v[:n, :], b2bc[:n, :],
                                            op=mybir.AluOpType.add)
                nc.sync.dma_start(out_p[w * WS:w * WS + n, :], zt[:n, :])

            agg_layer((None, U2a[:], U2b[:]), T2, so2, to2, OUT, u2res,
                      l2_cb, mid_hook=l2_mid)

    return nc


def run(cfg, inputs, sim=False, trace=False):
    from concourse.bass_utils import run_bass_kernel_spmd

    in_maps, T1, T2, b1nz, b2nz = _prepare(
        cfg, inputs["x"], inputs["edge_index"], inputs["W1"], inputs["b1"],
        inputs["W2"], inputs["b2"])
    nc = build_program(cfg, T1, T2, b1nz, b2nz)
    nc.finalize()
    core_ids = list(range(cfg.P))
    if sim:
        from concourse import bass_interp
        ms = bass_interp.MultiCoreSim(nc, cfg.P)
        for c in core_ids:
            for k, v in in_maps[c].items():
                ms.cores[c].tensor(k)[:] = v
        ms.simulate()
        outs = [np.array(ms.cores[c].tensor("out")) for c in core_ids]
        return np.concatenate(outs, axis=0), None
    res = run_bass_kernel_spmd(nc, in_maps, core_ids, trace=trace)
    outs = [np.asarray(res.results[c]["out"]) for c in core_ids]
    return np.concatenate(outs, axis=0), res


def kernel(x, edge_index, W1, b1, W2, b2):
    out, _ = run(FULL, dict(x=x, edge_index=edge_index, W1=W1, b1=b1,
                            W2=W2, b2=b2))
    return out
